# revision 13
# baseline (speedup 1.0000x reference)
"""Trainium2 Bass kernel for nn_MultiHeadAttention_60559038873660.

Reference math (faithful to the source bug: attention is contracted with the
projected K, not V, so v/Wv are dead inputs):
    qp = q @ Wq.T ; kp = k @ Wk.T
    head split via reshape(b, l, 64, 16): head n takes strided columns {d*16+n}
    S = Qh @ Kh.T / 8 ; A = softmax(S, axis=m) ; X = A @ Kh ; out = X @ Wo.T

Strategy (v2 — strip-major, fully pipelined):
  - Host-side: permute weight rows/cols head-major so each head is a contiguous
    64-column block; pre-transpose q/k/weights into the layouts the TensorE
    wants (contraction on partitions).
  - 8 cores = 2 batches x 4 head-groups (4 heads each).  Each core computes its
    4 heads' attention plus a partial output projection; the host sums the 4
    partials per batch (tensor-parallel row-split reduction).
  - The per-core schedule is ScalarE-bound (softmax exp = 16.8M elem at
    1 elem/lane/cycle @1.2GHz ~ 147us).  Everything else hides under it:
      * strip-major order (l-strip outer, head-pair group inner) so the output
        projection + stores stream per strip instead of piling up at the end
      * attention starts ~11us in, after only {Wq, q-strip0, Wk, k-tile0} DMAs
        and the group-0 projections of those tiles; all remaining projection /
        transpose / out-projection work is pumped into the attention loop as
        fine-grained fillers between iterations (PE has ~40% slack vs ACT)
      * softmax denominators ride the X^T matmul as a fused ones-column (row
        64); normalization = DVE reciprocal + SBUF->SBUF DMA partition
        broadcast + fused multiply during the PSUM->SBUF drain (no DRAM trip)
      * PSUM: 4 banks score double-buffer, 2 banks X accum, 2 banks shared
        aux ring (projection / out-projection / transpose targets)
"""

import contextlib
import ctypes
import os
import sys
import types
from collections import deque

import numpy as np

import concourse.bacc as bacc
import concourse.tile as tile
from concourse import mybir
from concourse.bass import ds, ts
from concourse.bass_utils import run_bass_kernel_spmd


def _install_ntff_hook():
    """Provide antenv.axon_hooks if the image lacks it, wiring NTFF
    profiling straight into libaxon_pjrt.so (same ABI trn_boot uses)."""
    try:
        import antenv.axon_hooks  # noqa: F401
        return
    except ImportError:
        pass
    mod = types.ModuleType("antenv.axon_hooks")
    holder = [None]
    mod.set_axon_ntff_profile_hook = lambda h: holder.__setitem__(0, h)
    mod.get_axon_ntff_profile_hook = lambda: holder[0]
    sys.modules["antenv.axon_hooks"] = mod
    try:
        import antenv
        antenv.axon_hooks = mod
    except ImportError:
        pass

    so_path = "/opt/axon/libaxon_pjrt.so"
    if not os.path.exists(so_path):
        return
    lib = ctypes.CDLL(so_path)
    if not hasattr(lib, "axon_start_nrt_profile"):
        return
    lib.axon_start_nrt_profile.argtypes = [ctypes.POINTER(ctypes.c_int64), ctypes.c_size_t]
    lib.axon_start_nrt_profile.restype = ctypes.c_int64
    lib.axon_stop_nrt_profile.argtypes = [ctypes.c_char_p]
    lib.axon_stop_nrt_profile.restype = ctypes.c_int64

    @contextlib.contextmanager
    def _hook(output_dir, device_ids):
        import jax
        jax.devices()
        if device_ids:
            ids = (ctypes.c_int64 * len(device_ids))(*device_ids)
            rc = lib.axon_start_nrt_profile(ids, len(device_ids))
        else:
            rc = lib.axon_start_nrt_profile(None, 0)
        if rc != 0:
            raise RuntimeError(f"axon_start_nrt_profile rc={rc}")
        try:
            yield
        finally:
            n = lib.axon_stop_nrt_profile(str(output_dir).encode())
            print(f"profile: {n} file(s) written to {output_dir}", file=sys.stderr)

    mod.set_axon_ntff_profile_hook(_hook)


_install_ntff_hook()

f32 = mybir.dt.float32
f32r = mybir.dt.float32r
bf16 = mybir.dt.bfloat16
Exp = mybir.ActivationFunctionType.Exp

P = 128
DIM = 1024
NH = 16
HD = 64
HPC = 4          # heads per core
CW = HPC * HD    # 256 channel columns per core
CH = HD + 1      # head channels + ones column
G = CW // P      # 2 channel groups of 128
KC = DIM // P    # 8 contraction chunks for projections
JT = DIM // 512  # out-projection j tiles

_cache = {}


def _build(L, M):
    NT = 512                 # l-strip width / matmul moving tile
    L5 = L // NT             # 4 l-strips
    KTN = M // NT            # 4 k DMA tiles
    MG = M // P              # 16 m chunks per strip
    LC = L // P              # 16 out-proj l chunks

    nc = bacc.Bacc()
    qT = nc.declare_dram_parameter("qT", [DIM, L], bf16, isOutput=False)
    kT = nc.declare_dram_parameter("kT", [DIM, M], bf16, isOutput=False)
    wqT = nc.declare_dram_parameter("wqT", [DIM, CW], bf16, isOutput=False)
    wkT = nc.declare_dram_parameter("wkT", [DIM, CW], bf16, isOutput=False)
    woT = nc.declare_dram_parameter("woT", [CW, DIM], bf16, isOutput=False)
    out = nc.declare_dram_parameter("out", [L, DIM], f32, isOutput=True)
    rd_dram = nc.dram_tensor("rden_scratch", [HPC, L], f32)

    from concourse.masks import make_identity

    with tile.TileContext(nc) as tc:
        with (
            tc.tile_pool(name="singles", bufs=1) as singles,
            tc.tile_pool(name="qio", bufs=2) as qio,
            tc.tile_pool(name="es", bufs=4) as es_pool,
            tc.tile_pool(name="nrm", bufs=2) as nrm,
            tc.tile_pool(name="ost", bufs=3) as ost,
            tc.tile_pool(name="psS", bufs=2, space="PSUM") as psS,
            tc.tile_pool(name="psX", bufs=2, space="PSUM") as psX,
            tc.tile_pool(name="psA", bufs=2, space="PSUM") as psA,
        ):
            # ---- input DMAs: minimal prefix first (wq, q0, wk, k0..3) ----
            wq_sb = singles.tile([P, KC, CW], bf16)
            nc.sync.dma_start(wq_sb, wqT.rearrange("(kc p) c -> p kc c", p=P))
            # q strip tiles ride a 2-slot ring; allocated at DMA-emission
            # time, closures resolve them through this dict at pop time
            qin = {}

            def load_q(t):
                qin[t] = qio.tile([P, KC, NT], bf16, tag="qin", name=f"qin{t}")
                nc.sync.dma_start(
                    qin[t], qT[:, ts(t, NT)].rearrange("(kc p) l -> p kc l", p=P))

            load_q(0)
            wk_sb = singles.tile([P, KC, CW], bf16)
            nc.sync.dma_start(wk_sb, wkT.rearrange("(kc p) c -> p kc c", p=P))
            kin = []
            for t in range(KTN):
                kin_t = singles.tile([P, KC, NT], bf16, name=f"kin{t}")
                nc.sync.dma_start(
                    kin_t, kT[:, ts(t, NT)].rearrange("(kc p) l -> p kc l", p=P))
                kin.append(kin_t)
            load_q(1)
            wo_sb = singles.tile([P, G, DIM], bf16)
            nc.sync.dma_start(wo_sb, woT.rearrange("(g p) j -> p g j", p=P))

            qhT = singles.tile([P, G, L], bf16)
            khT = singles.tile([P, G, M], bf16)
            khp = singles.tile([P, MG, HPC, CH], bf16)
            xu = singles.tile([P, G, L], bf16)
            ident = singles.tile([P, P], bf16)
            make_identity(nc, ident)

            ones_sb = singles.tile([P, 1], f32)
            nc.vector.memset(ones_sb, 1.0)
            for mg in range(MG):
                nc.vector.tensor_copy(khp[:, mg, :, HD:CH],
                                      ones_sb[:, None, :].to_broadcast([P, HPC, 1]))

            # ---- filler units (each <= ~2 matmuls of PE work) ----
            def proj_units(dst, w_sb, src, tt, g):
                """q/k projection of one (strip, group): 4 units x 2 MMs.
                src is a thunk resolved at pop time (q tiles alloc late)."""
                st = {}

                def unit(k, st=st):
                    if k == 0:
                        st["ps"] = psA.tile([P, NT], f32, tag="aux", name="pps")
                    ps = st["ps"]
                    src_t = src()
                    for kc in (2 * k, 2 * k + 1):
                        nc.tensor.matmul(ps, lhsT=w_sb[:, kc, ts(g, P)],
                                         rhs=src_t[:, kc],
                                         start=(kc == 0), stop=(kc == KC - 1))
                    if k == 3:
                        nc.vector.tensor_copy(dst[:, g, ts(tt, NT)], ps)

                return [lambda k=k: unit(k) for k in range(4)]

            def ktrans_unit(mc, g):
                def unit():
                    tr = psA.tile([P, P], bf16, tag="aux", name="trp")
                    nc.tensor.transpose(tr, khT[:, g, ts(mc, P)], ident)
                    for hh in range(2):
                        nc.vector.tensor_copy(khp[:, mc, g * 2 + hh, 0:HD],
                                              tr[:, ts(hh, HD)])
                return [unit]

            def po_unit(lc, jt):
                def unit():
                    po = psA.tile([P, 512], f32, tag="aux", name="pop")
                    for cc in range(G):
                        nc.tensor.matmul(po, lhsT=xu[:, cc, ts(lc, P)],
                                         rhs=wo_sb[:, cc, ts(jt, 512)],
                                         start=(cc == 0), stop=(cc == G - 1))
                    ot = ost.tile([P, 512], f32, tag="ot")
                    nc.vector.tensor_copy(ot, po)
                    nc.gpsimd.dma_start(out[ts(lc, P), ts(jt, 512)], ot)
                return [unit]

            def q_src(t):
                return lambda: qin[t]

            def k_src(t):
                return lambda: kin[t]

            # ---- head phase: projections feeding sub-strip (0, g0) ----
            for g in range(G):
                for u in proj_units(qhT, wq_sb, q_src(0), 0, g):
                    u()
            for u in proj_units(khT, wk_sb, k_src(0), 0, 0):
                u()
            for mc in range(4):
                for u in ktrans_unit(mc, 0):
                    u()

            # ---- per-sub-strip filler queues ----
            def kproj_strip_fill(g, qp_after):
                """kproj m-tiles 1..3 (+m0 of g1) & ktrans, deadline-ordered."""
                fs = []
                fs += proj_units(khT, wk_sb, k_src(1), 1, g)
                fs += ktrans_unit(4, g) + ktrans_unit(5, g)
                fs += proj_units(khT, wk_sb, k_src(2), 2, g)
                for mc in range(6, 10):
                    fs += ktrans_unit(mc, g)
                fs += proj_units(khT, wk_sb, k_src(3), 3, g)
                for mc in range(10, MG):
                    fs += ktrans_unit(mc, g)
                fs += qp_after
                return deque(fs)

            fill = {}
            fill[(0, 0)] = kproj_strip_fill(
                0, proj_units(khT, wk_sb, k_src(0), 0, 1) +
                [u for mc in range(4) for u in ktrans_unit(mc, 1)])
            fill[(0, 1)] = kproj_strip_fill(1, proj_units(qhT, wq_sb, q_src(1), 1, 0))
            for l5 in range(1, L5):
                fs = deque(proj_units(qhT, wq_sb, q_src(l5), l5, 1))
                for lc in range(4 * (l5 - 1), 4 * l5):
                    for jt in range(JT):
                        fs += po_unit(lc, jt)
                fill[(l5, 0)] = fs
                if l5 < L5 - 1:
                    fill[(l5, 1)] = deque(
                        proj_units(qhT, wq_sb, q_src(l5 + 1), l5 + 1, 0))
                else:
                    fill[(l5, 1)] = deque()
            budget = {k: (2 if k[0] == 0 else 1) for k in fill}

            # ---- main loop: strip-major attention with woven fillers ----
            for l5 in range(L5):
                lsl = ts(l5, NT)
                # prefetch next q strip into the freed qio slot
                if 1 <= l5 < L5 - 1:
                    load_q(l5 + 1)
                for g in range(G):
                    hA, hB = 2 * g, 2 * g + 1
                    fq, nb = fill[(l5, g)], budget[(l5, g)]

                    def emit_sp(mc, g=g, lsl=lsl):
                        sps = psS.tile([P, 2 * NT], f32, tag="s")
                        nc.tensor.matmul(sps[:, 0:NT],
                                         lhsT=khT[0:HD, g, ts(mc, P)],
                                         rhs=qhT[0:HD, g, lsl],
                                         start=True, stop=True)
                        nc.tensor.matmul(sps[:, NT:2 * NT],
                                         lhsT=khT[HD:P, g, ts(mc, P)],
                                         rhs=qhT[HD:P, g, lsl],
                                         start=True, stop=True)
                        return sps

                    xpsA = psX.tile([CH, NT], f32, tag="x")
                    xpsB = psX.tile([CH, NT], f32, tag="x")
                    sq = [emit_sp(0), emit_sp(1)]
                    for mc in range(MG):
                        for _ in range(min(nb, len(fq))):
                            fq.popleft()()
                        if mc + 2 < MG:
                            sq.append(emit_sp(mc + 2))
                        es = es_pool.tile([P, 2 * NT], bf16, tag="es")
                        nc.scalar.activation(es, sq.pop(0), Exp, scale=0.125)
                        nc.tensor.matmul(xpsA, lhsT=khp[:, mc, hA, :],
                                         rhs=es[:, 0:NT],
                                         start=(mc == 0), stop=(mc == MG - 1))
                        nc.tensor.matmul(xpsB, lhsT=khp[:, mc, hB, :],
                                         rhs=es[:, NT:2 * NT],
                                         start=(mc == 0), stop=(mc == MG - 1))
                    while fq:
                        fq.popleft()()

                    # drain X accumulators fast (frees PSUM), then normalize:
                    # rden = 1/row64 ; xu = X * rden  (SBUF->SBUF bcast DMA)
                    xrs = []
                    for xps in (xpsA, xpsB):
                        xr = nrm.tile([CH, NT], f32, tag="xr", name="xr")
                        nc.vector.tensor_copy(xr, xps)
                        xrs.append(xr)
                    for hh, xr in enumerate(xrs):
                        # raw denominator row -> DRAM -> partition-broadcast;
                        # reciprocal runs wide ([64,NT], not [1,NT]: 6x faster)
                        h = 2 * g + hh
                        nc.gpsimd.dma_start(rd_dram[h:h + 1, lsl], xr[HD:CH])
                        dbc = nrm.tile([HD, NT], f32, tag="dbc", name="dbc")
                        nc.gpsimd.dma_start(
                            dbc, rd_dram[h:h + 1, lsl].to_broadcast([HD, NT]))
                        rdbc = nrm.tile([HD, NT], f32, tag="rdbc", name="rdbc")
                        nc.vector.reciprocal(rdbc, dbc)
                        nc.vector.tensor_mul(xu[ts(hh, HD), g, lsl],
                                             xr[0:HD], rdbc)

            # ---- tail: out-projection of the last strip ----
            for lc in range(4 * (L5 - 1), LC):
                for jt in range(JT):
                    for u in po_unit(lc, jt):
                        u()

    nc.finalize()
    return nc


def _get_nc(L, M):
    key = (L, M)
    if key not in _cache:
        _cache[key] = _build(L, M)
    return _cache[key]


# head-major channel permutation: new channel c = h*64+d <- original column d*16+h
_PERM = np.array([(c % HD) * NH + c // HD for c in range(DIM)])

last_exec_time_ns = None
last_results = None


def kernel(q, k, v, Wq, Wk, Wv, Wo):  # noqa: ARG001 - v/Wv dead in reference
    global last_exec_time_ns, last_results
    q = np.asarray(q, np.float32)
    k = np.asarray(k, np.float32)
    Wq = np.asarray(Wq, np.float32)
    Wk = np.asarray(Wk, np.float32)
    Wo = np.asarray(Wo, np.float32)
    B, L, _ = q.shape
    M = k.shape[1]

    import ml_dtypes
    bf = ml_dtypes.bfloat16
    Wq_p = Wq[_PERM]            # (1024, 1024) head-major rows
    Wk_p = Wk[_PERM]
    WoT_p = Wo[:, _PERM].T      # (1024 c, 1024 j)

    qT = [np.ascontiguousarray(q[b].T).astype(bf) for b in range(B)]
    kT = [np.ascontiguousarray(k[b].T).astype(bf) for b in range(B)]
    wqT = [np.ascontiguousarray(Wq_p[hg * CW:(hg + 1) * CW, :].T).astype(bf) for hg in range(4)]
    wkT = [np.ascontiguousarray(Wk_p[hg * CW:(hg + 1) * CW, :].T).astype(bf) for hg in range(4)]
    woT = [np.ascontiguousarray(WoT_p[hg * CW:(hg + 1) * CW, :]).astype(bf) for hg in range(4)]

    in_maps = []
    for core in range(8):
        b, hg = divmod(core, 4)
        in_maps.append({"qT": qT[b], "kT": kT[b], "wqT": wqT[hg],
                        "wkT": wkT[hg], "woT": woT[hg]})

    nc = _get_nc(L, M)
    trace = bool(int(os.environ.get("MHA_TRACE", "0")))
    res = run_bass_kernel_spmd(nc, in_maps, core_ids=list(range(8)), trace=trace)
    last_results = res
    last_exec_time_ns = res.exec_time_ns

    out = np.zeros((B, L, DIM), np.float32)
    for core in range(8):
        b = core // 4
        out[b] += res.results[core]["out"]
    return out


# revision 17
# speedup vs baseline: 1.0584x; 1.0584x over previous
"""Trainium2 Bass kernel for nn_MultiHeadAttention_60559038873660.

Reference math (faithful to the source bug: attention is contracted with the
projected K, not V, so v/Wv are dead inputs):
    qp = q @ Wq.T ; kp = k @ Wk.T
    head split via reshape(b, l, 64, 16): head n takes strided columns {d*16+n}
    S = Qh @ Kh.T / 8 ; A = softmax(S, axis=m) ; X = A @ Kh ; out = X @ Wo.T

Strategy (v2 — strip-major, fully pipelined):
  - Host-side: permute weight rows/cols head-major so each head is a contiguous
    64-column block; pre-transpose q/k/weights into the layouts the TensorE
    wants (contraction on partitions).
  - 8 cores = 2 batches x 4 head-groups (4 heads each).  Each core computes its
    4 heads' attention plus a partial output projection; the host sums the 4
    partials per batch (tensor-parallel row-split reduction).
  - The per-core schedule is ScalarE-bound (softmax exp = 16.8M elem at
    1 elem/lane/cycle @1.2GHz ~ 147us).  Everything else hides under it:
      * strip-major order (l-strip outer, head-pair group inner) so the output
        projection + stores stream per strip instead of piling up at the end
      * attention starts ~11us in, after only {Wq, q-strip0, Wk, k-tile0} DMAs
        and the group-0 projections of those tiles; all remaining projection /
        transpose / out-projection work is pumped into the attention loop as
        fine-grained fillers between iterations (PE has ~40% slack vs ACT)
      * softmax denominators ride the X^T matmul as a fused ones-column (row
        64); normalization = DVE reciprocal + SBUF->SBUF DMA partition
        broadcast + fused multiply during the PSUM->SBUF drain (no DRAM trip)
      * PSUM: 4 banks score double-buffer, 2 banks X accum, 2 banks shared
        aux ring (projection / out-projection / transpose targets)
"""

import contextlib
import ctypes
import os
import sys
import types
from collections import deque

import numpy as np

import concourse.bacc as bacc
import concourse.tile as tile
from concourse import mybir
from concourse.bass import ds, ts
from concourse.bass_utils import run_bass_kernel_spmd


def _install_ntff_hook():
    """Provide antenv.axon_hooks if the image lacks it, wiring NTFF
    profiling straight into libaxon_pjrt.so (same ABI trn_boot uses)."""
    try:
        import antenv.axon_hooks  # noqa: F401
        return
    except ImportError:
        pass
    mod = types.ModuleType("antenv.axon_hooks")
    holder = [None]
    mod.set_axon_ntff_profile_hook = lambda h: holder.__setitem__(0, h)
    mod.get_axon_ntff_profile_hook = lambda: holder[0]
    sys.modules["antenv.axon_hooks"] = mod
    try:
        import antenv
        antenv.axon_hooks = mod
    except ImportError:
        pass

    so_path = "/opt/axon/libaxon_pjrt.so"
    if not os.path.exists(so_path):
        return
    lib = ctypes.CDLL(so_path)
    if not hasattr(lib, "axon_start_nrt_profile"):
        return
    lib.axon_start_nrt_profile.argtypes = [ctypes.POINTER(ctypes.c_int64), ctypes.c_size_t]
    lib.axon_start_nrt_profile.restype = ctypes.c_int64
    lib.axon_stop_nrt_profile.argtypes = [ctypes.c_char_p]
    lib.axon_stop_nrt_profile.restype = ctypes.c_int64

    @contextlib.contextmanager
    def _hook(output_dir, device_ids):
        import jax
        jax.devices()
        if device_ids:
            ids = (ctypes.c_int64 * len(device_ids))(*device_ids)
            rc = lib.axon_start_nrt_profile(ids, len(device_ids))
        else:
            rc = lib.axon_start_nrt_profile(None, 0)
        if rc != 0:
            raise RuntimeError(f"axon_start_nrt_profile rc={rc}")
        try:
            yield
        finally:
            n = lib.axon_stop_nrt_profile(str(output_dir).encode())
            print(f"profile: {n} file(s) written to {output_dir}", file=sys.stderr)

    mod.set_axon_ntff_profile_hook(_hook)


_install_ntff_hook()

f32 = mybir.dt.float32
f32r = mybir.dt.float32r
bf16 = mybir.dt.bfloat16
Exp = mybir.ActivationFunctionType.Exp

P = 128
DIM = 1024
NH = 16
HD = 64
HPC = 4          # heads per core
CW = HPC * HD    # 256 channel columns per core
CH = HD + 1      # head channels + ones column
G = CW // P      # 2 channel groups of 128
KC = DIM // P    # 8 contraction chunks for projections
JT = DIM // 512  # out-projection j tiles

_cache = {}


def _build(L, M):
    NT = 512                 # l-strip width / matmul moving tile
    L5 = L // NT             # 4 l-strips
    KTN = M // NT            # 4 k DMA tiles
    MG = M // P              # 16 m chunks per strip
    LC = L // P              # 16 out-proj l chunks

    nc = bacc.Bacc()
    qT = nc.declare_dram_parameter("qT", [DIM, L], bf16, isOutput=False)
    kT = nc.declare_dram_parameter("kT", [DIM, M], bf16, isOutput=False)
    wqT = nc.declare_dram_parameter("wqT", [DIM, CW], bf16, isOutput=False)
    wkT = nc.declare_dram_parameter("wkT", [DIM, CW], bf16, isOutput=False)
    woT = nc.declare_dram_parameter("woT", [CW, DIM], bf16, isOutput=False)
    out = nc.declare_dram_parameter("out", [L, DIM], f32, isOutput=True)
    rd_dram = nc.dram_tensor("rden_scratch", [HPC, L], f32)

    from concourse.masks import make_identity

    with tile.TileContext(nc) as tc:
        with (
            tc.tile_pool(name="singles", bufs=1) as singles,
            tc.tile_pool(name="qio", bufs=2) as qio,
            tc.tile_pool(name="es", bufs=4) as es_pool,
            tc.tile_pool(name="nrm", bufs=2) as nrm,
            tc.tile_pool(name="ost", bufs=3) as ost,
            tc.tile_pool(name="psS", bufs=2, space="PSUM") as psS,
            tc.tile_pool(name="psX", bufs=2, space="PSUM") as psX,
            tc.tile_pool(name="psA", bufs=2, space="PSUM") as psA,
        ):
            # ---- input DMAs: minimal prefix first (wq, q0, wk, k0..3) ----
            # q-side loads ride the scalar (ACT) DMA queue at startup so the
            # q and k input streams transfer in parallel; ACT is idle then
            wq_sb = singles.tile([P, KC, CW], bf16)
            nc.scalar.dma_start(wq_sb, wqT.rearrange("(kc p) c -> p kc c", p=P))
            # q strip tiles ride a 2-slot ring; allocated at DMA-emission
            # time, closures resolve them through this dict at pop time
            qin = {}

            def load_q(t, eng=None):
                qin[t] = qio.tile([P, KC, NT], bf16, tag="qin", name=f"qin{t}")
                (eng or nc.sync).dma_start(
                    qin[t], qT[:, ts(t, NT)].rearrange("(kc p) l -> p kc l", p=P))

            load_q(0, nc.scalar)
            wk_sb = singles.tile([P, KC, CW], bf16)
            nc.sync.dma_start(wk_sb, wkT.rearrange("(kc p) c -> p kc c", p=P))
            kin = []
            for t in range(KTN):
                kin_t = singles.tile([P, KC, NT], bf16, name=f"kin{t}")
                nc.sync.dma_start(
                    kin_t, kT[:, ts(t, NT)].rearrange("(kc p) l -> p kc l", p=P))
                kin.append(kin_t)
            load_q(1)
            wo_sb = singles.tile([P, G, DIM], bf16)
            nc.sync.dma_start(wo_sb, woT.rearrange("(g p) j -> p g j", p=P))

            qhT = singles.tile([P, G, L], bf16)
            khT = singles.tile([P, G, M], bf16)
            khp = singles.tile([P, MG, HPC, CH], bf16)
            xu = singles.tile([P, G, L], bf16)
            ident = singles.tile([P, P], bf16)
            make_identity(nc, ident)

            ones_sb = singles.tile([P, 1], f32)
            nc.vector.memset(ones_sb, 1.0)
            for mg in range(MG):
                nc.vector.tensor_copy(khp[:, mg, :, HD:CH],
                                      ones_sb[:, None, :].to_broadcast([P, HPC, 1]))

            # ---- filler units (each <= ~2 matmuls of PE work) ----
            def proj_units(dst, w_sb, src, tt, g):
                """q/k projection of one (strip, group): 4 units x 2 MMs.
                src is a thunk resolved at pop time (q tiles alloc late)."""
                st = {}

                def unit(k, st=st):
                    if k == 0:
                        st["ps"] = psA.tile([P, NT], f32, tag="aux", name="pps")
                    ps = st["ps"]
                    src_t = src()
                    for kc in (2 * k, 2 * k + 1):
                        nc.tensor.matmul(ps, lhsT=w_sb[:, kc, ts(g, P)],
                                         rhs=src_t[:, kc],
                                         start=(kc == 0), stop=(kc == KC - 1))
                    if k == 3:
                        nc.vector.tensor_copy(dst[:, g, ts(tt, NT)], ps)

                return [lambda k=k: unit(k) for k in range(4)]

            def ktrans_unit(mc, g):
                def unit():
                    tr = psA.tile([P, P], bf16, tag="aux", name="trp")
                    nc.tensor.transpose(tr, khT[:, g, ts(mc, P)], ident)
                    for hh in range(2):
                        nc.vector.tensor_copy(khp[:, mc, g * 2 + hh, 0:HD],
                                              tr[:, ts(hh, HD)])
                return [unit]

            def po_unit(lc, jt):
                def unit():
                    po = psA.tile([P, 512], f32, tag="aux", name="pop")
                    for cc in range(G):
                        nc.tensor.matmul(po, lhsT=xu[:, cc, ts(lc, P)],
                                         rhs=wo_sb[:, cc, ts(jt, 512)],
                                         start=(cc == 0), stop=(cc == G - 1))
                    ot = ost.tile([P, 512], f32, tag="ot")
                    nc.vector.tensor_copy(ot, po)
                    # stores ride sync, keeping gpsimd free for the
                    # latency-sensitive normalization round trips
                    nc.sync.dma_start(out[ts(lc, P), ts(jt, 512)], ot)
                return [unit]

            def q_src(t):
                return lambda: qin[t]

            def k_src(t):
                return lambda: kin[t]

            # ---- head phase: projections feeding sub-strip (0, g0) ----
            for g in range(G):
                for u in proj_units(qhT, wq_sb, q_src(0), 0, g):
                    u()
            for u in proj_units(khT, wk_sb, k_src(0), 0, 0):
                u()
            for mc in range(4):
                for u in ktrans_unit(mc, 0):
                    u()

            # ---- per-sub-strip filler queues ----
            def kproj_strip_fill(g, qp_after):
                """kproj m-tiles 1..3 (+m0 of g1) & ktrans, deadline-ordered."""
                fs = []
                fs += proj_units(khT, wk_sb, k_src(1), 1, g)
                fs += ktrans_unit(4, g) + ktrans_unit(5, g)
                fs += proj_units(khT, wk_sb, k_src(2), 2, g)
                for mc in range(6, 10):
                    fs += ktrans_unit(mc, g)
                fs += proj_units(khT, wk_sb, k_src(3), 3, g)
                for mc in range(10, MG):
                    fs += ktrans_unit(mc, g)
                fs += qp_after
                return deque(fs)

            fill = {}
            fill[(0, 0)] = kproj_strip_fill(
                0, proj_units(khT, wk_sb, k_src(0), 0, 1) +
                [u for mc in range(4) for u in ktrans_unit(mc, 1)])
            fill[(0, 1)] = kproj_strip_fill(1, proj_units(qhT, wq_sb, q_src(1), 1, 0))
            for l5 in range(1, L5):
                fs = deque(proj_units(qhT, wq_sb, q_src(l5), l5, 1))
                fs += [lambda: None, lambda: None]  # let prior strip's xu land
                for lc in range(4 * (l5 - 1), 4 * l5):
                    for jt in range(JT):
                        fs += po_unit(lc, jt)
                fill[(l5, 0)] = fs
                if l5 < L5 - 1:
                    fill[(l5, 1)] = deque(
                        proj_units(qhT, wq_sb, q_src(l5 + 1), l5 + 1, 0))
                else:
                    fill[(l5, 1)] = deque()
            budget = {k: (2 if k[0] == 0 else 1) for k in fill}

            # ---- main loop: strip-major attention with woven fillers ----
            for l5 in range(L5):
                lsl = ts(l5, NT)
                # prefetch next q strip into the freed qio slot
                if 1 <= l5 < L5 - 1:
                    load_q(l5 + 1)
                for g in range(G):
                    hA, hB = 2 * g, 2 * g + 1
                    fq, nb = fill[(l5, g)], budget[(l5, g)]

                    def emit_sp(mc, g=g, lsl=lsl):
                        sps = psS.tile([P, 2 * NT], f32, tag="s")
                        nc.tensor.matmul(sps[:, 0:NT],
                                         lhsT=khT[0:HD, g, ts(mc, P)],
                                         rhs=qhT[0:HD, g, lsl],
                                         start=True, stop=True)
                        nc.tensor.matmul(sps[:, NT:2 * NT],
                                         lhsT=khT[HD:P, g, ts(mc, P)],
                                         rhs=qhT[HD:P, g, lsl],
                                         start=True, stop=True)
                        return sps

                    xpsA = psX.tile([CH, NT], f32, tag="x")
                    xpsB = psX.tile([CH, NT], f32, tag="x")
                    sq = [emit_sp(0), emit_sp(1)]
                    for mc in range(MG):
                        for _ in range(min(nb, len(fq))):
                            fq.popleft()()
                        if mc + 2 < MG:
                            sq.append(emit_sp(mc + 2))
                        es = es_pool.tile([P, 2 * NT], bf16, tag="es")
                        nc.scalar.activation(es, sq.pop(0), Exp, scale=0.125)
                        nc.tensor.matmul(xpsA, lhsT=khp[:, mc, hA, :],
                                         rhs=es[:, 0:NT],
                                         start=(mc == 0), stop=(mc == MG - 1))
                        nc.tensor.matmul(xpsB, lhsT=khp[:, mc, hB, :],
                                         rhs=es[:, NT:2 * NT],
                                         start=(mc == 0), stop=(mc == MG - 1))
                    while fq:
                        fq.popleft()()

                    # drain X accumulators fast (frees PSUM), then normalize:
                    # rden = 1/row64 ; xu = X * rden  (SBUF->SBUF bcast DMA)
                    xrs = []
                    for xps in (xpsA, xpsB):
                        xr = nrm.tile([CH, NT], f32, tag="xr", name="xr")
                        nc.vector.tensor_copy(xr, xps)
                        xrs.append(xr)
                    for hh, xr in enumerate(xrs):
                        # raw denominator row -> DRAM -> partition-broadcast;
                        # reciprocal runs wide ([64,NT], not [1,NT]: 6x faster)
                        h = 2 * g + hh
                        nc.gpsimd.dma_start(rd_dram[h:h + 1, lsl], xr[HD:CH])
                        dbc = nrm.tile([HD, NT], f32, tag="dbc", name="dbc")
                        nc.gpsimd.dma_start(
                            dbc, rd_dram[h:h + 1, lsl].to_broadcast([HD, NT]))
                        rdbc = nrm.tile([HD, NT], f32, tag="rdbc", name="rdbc")
                        nc.vector.reciprocal_approx_fast(rdbc, dbc)
                        nc.vector.tensor_mul(xu[ts(hh, HD), g, lsl],
                                             xr[0:HD], rdbc)

            # ---- tail: out-projection of the last strip ----
            for lc in range(4 * (L5 - 1), LC):
                for jt in range(JT):
                    for u in po_unit(lc, jt):
                        u()

    nc.finalize()
    return nc


def _get_nc(L, M):
    key = (L, M)
    if key not in _cache:
        _cache[key] = _build(L, M)
    return _cache[key]


# head-major channel permutation: new channel c = h*64+d <- original column d*16+h
_PERM = np.array([(c % HD) * NH + c // HD for c in range(DIM)])

last_exec_time_ns = None
last_results = None


def kernel(q, k, v, Wq, Wk, Wv, Wo):  # noqa: ARG001 - v/Wv dead in reference
    global last_exec_time_ns, last_results
    q = np.asarray(q, np.float32)
    k = np.asarray(k, np.float32)
    Wq = np.asarray(Wq, np.float32)
    Wk = np.asarray(Wk, np.float32)
    Wo = np.asarray(Wo, np.float32)
    B, L, _ = q.shape
    M = k.shape[1]

    import ml_dtypes
    bf = ml_dtypes.bfloat16
    Wq_p = Wq[_PERM]            # (1024, 1024) head-major rows
    Wk_p = Wk[_PERM]
    WoT_p = Wo[:, _PERM].T      # (1024 c, 1024 j)

    qT = [np.ascontiguousarray(q[b].T).astype(bf) for b in range(B)]
    kT = [np.ascontiguousarray(k[b].T).astype(bf) for b in range(B)]
    wqT = [np.ascontiguousarray(Wq_p[hg * CW:(hg + 1) * CW, :].T).astype(bf) for hg in range(4)]
    wkT = [np.ascontiguousarray(Wk_p[hg * CW:(hg + 1) * CW, :].T).astype(bf) for hg in range(4)]
    woT = [np.ascontiguousarray(WoT_p[hg * CW:(hg + 1) * CW, :]).astype(bf) for hg in range(4)]

    in_maps = []
    for core in range(8):
        b, hg = divmod(core, 4)
        in_maps.append({"qT": qT[b], "kT": kT[b], "wqT": wqT[hg],
                        "wkT": wkT[hg], "woT": woT[hg]})

    nc = _get_nc(L, M)
    trace = bool(int(os.environ.get("MHA_TRACE", "0")))
    res = run_bass_kernel_spmd(nc, in_maps, core_ids=list(range(8)), trace=trace)
    last_results = res
    last_exec_time_ns = res.exec_time_ns

    out = np.zeros((B, L, DIM), np.float32)
    for core in range(8):
        b = core // 4
        out[b] += res.results[core]["out"]
    return out


# revision 22
# speedup vs baseline: 1.2752x; 1.2048x over previous
"""Trainium2 Bass kernel for nn_MultiHeadAttention_60559038873660.

Reference math (faithful to the source bug: attention is contracted with the
projected K, not V, so v/Wv are dead inputs):
    qp = q @ Wq.T ; kp = k @ Wk.T
    head split via reshape(b, l, 64, 16): head n takes strided columns {d*16+n}
    S = Qh @ Kh.T / 8 ; A = softmax(S, axis=m) ; X = A @ Kh ; out = X @ Wo.T

Strategy (v2 — strip-major, fully pipelined):
  - Host-side: permute weight rows/cols head-major so each head is a contiguous
    64-column block; pre-transpose q/k/weights into the layouts the TensorE
    wants (contraction on partitions).
  - 8 cores = 2 batches x 4 head-groups (4 heads each).  Each core computes its
    4 heads' attention plus a partial output projection; the host sums the 4
    partials per batch (tensor-parallel row-split reduction).
  - The per-core schedule is ScalarE-bound (softmax exp = 16.8M elem at
    1 elem/lane/cycle @1.2GHz ~ 147us).  Everything else hides under it:
      * strip-major order (l-strip outer, head-pair group inner) so the output
        projection + stores stream per strip instead of piling up at the end
      * attention starts ~11us in, after only {Wq, q-strip0, Wk, k-tile0} DMAs
        and the group-0 projections of those tiles; all remaining projection /
        transpose / out-projection work is pumped into the attention loop as
        fine-grained fillers between iterations (PE has ~40% slack vs ACT)
      * softmax denominators ride the X^T matmul as a fused ones-column (row
        64); normalization = DVE reciprocal + SBUF->SBUF DMA partition
        broadcast + fused multiply during the PSUM->SBUF drain (no DRAM trip)
      * PSUM: 4 banks score double-buffer, 2 banks X accum, 2 banks shared
        aux ring (projection / out-projection / transpose targets)
"""

import contextlib
import ctypes
import os
import sys
import types
from collections import deque

import numpy as np

import concourse.bacc as bacc
import concourse.tile as tile
from concourse import mybir
from concourse.bass import ds, ts
from concourse.bass_utils import run_bass_kernel_spmd


def _install_ntff_hook():
    """Provide antenv.axon_hooks if the image lacks it, wiring NTFF
    profiling straight into libaxon_pjrt.so (same ABI trn_boot uses)."""
    try:
        import antenv.axon_hooks  # noqa: F401
        return
    except ImportError:
        pass
    mod = types.ModuleType("antenv.axon_hooks")
    holder = [None]
    mod.set_axon_ntff_profile_hook = lambda h: holder.__setitem__(0, h)
    mod.get_axon_ntff_profile_hook = lambda: holder[0]
    sys.modules["antenv.axon_hooks"] = mod
    try:
        import antenv
        antenv.axon_hooks = mod
    except ImportError:
        pass

    so_path = "/opt/axon/libaxon_pjrt.so"
    if not os.path.exists(so_path):
        return
    lib = ctypes.CDLL(so_path)
    if not hasattr(lib, "axon_start_nrt_profile"):
        return
    lib.axon_start_nrt_profile.argtypes = [ctypes.POINTER(ctypes.c_int64), ctypes.c_size_t]
    lib.axon_start_nrt_profile.restype = ctypes.c_int64
    lib.axon_stop_nrt_profile.argtypes = [ctypes.c_char_p]
    lib.axon_stop_nrt_profile.restype = ctypes.c_int64

    @contextlib.contextmanager
    def _hook(output_dir, device_ids):
        import jax
        jax.devices()
        if device_ids:
            ids = (ctypes.c_int64 * len(device_ids))(*device_ids)
            rc = lib.axon_start_nrt_profile(ids, len(device_ids))
        else:
            rc = lib.axon_start_nrt_profile(None, 0)
        if rc != 0:
            raise RuntimeError(f"axon_start_nrt_profile rc={rc}")
        try:
            yield
        finally:
            n = lib.axon_stop_nrt_profile(str(output_dir).encode())
            print(f"profile: {n} file(s) written to {output_dir}", file=sys.stderr)

    mod.set_axon_ntff_profile_hook(_hook)


_install_ntff_hook()

f32 = mybir.dt.float32
f32r = mybir.dt.float32r
bf16 = mybir.dt.bfloat16
Exp = mybir.ActivationFunctionType.Exp

P = 128
DIM = 1024
NH = 16
HD = 64
HPC = 4          # heads per core
CW = HPC * HD    # 256 channel columns per core
CH = HD + 1      # head channels + ones column
G = CW // P      # 2 channel groups of 128
KC = DIM // P    # 8 contraction chunks for projections
JT = DIM // 512  # out-projection j tiles

_cache = {}


def _build(L, M):
    NT = 512                 # l-strip width / matmul moving tile
    L5 = L // NT             # 4 l-strips
    KTN = M // NT            # 4 k DMA tiles
    MG = M // P              # 16 m chunks per strip
    LC = L // P              # 16 out-proj l chunks

    nc = bacc.Bacc()
    qT = nc.declare_dram_parameter("qT", [DIM, L], bf16, isOutput=False)
    kT = nc.declare_dram_parameter("kT", [DIM, M], bf16, isOutput=False)
    wqT = nc.declare_dram_parameter("wqT", [DIM, CW], bf16, isOutput=False)
    wkT = nc.declare_dram_parameter("wkT", [DIM, CW], bf16, isOutput=False)
    woT = nc.declare_dram_parameter("woT", [CW, DIM], bf16, isOutput=False)
    out = nc.declare_dram_parameter("out", [L, DIM], f32, isOutput=True)
    rd_dram = nc.dram_tensor("rden_scratch", [HPC, L], f32)

    from concourse.masks import make_identity

    with tile.TileContext(nc) as tc:
        with (
            tc.tile_pool(name="singles", bufs=1) as singles,
            tc.tile_pool(name="qio", bufs=2) as qio,
            tc.tile_pool(name="es", bufs=4) as es_pool,
            tc.tile_pool(name="nrm", bufs=2) as nrm,
            tc.tile_pool(name="ost", bufs=3) as ost,
            tc.tile_pool(name="psS", bufs=2, space="PSUM") as psS,
            tc.tile_pool(name="psX", bufs=2, space="PSUM") as psX,
            tc.tile_pool(name="psA", bufs=2, space="PSUM") as psA,
        ):
            # ---- input DMAs: minimal prefix first (wq, q0, wk, k0..3) ----
            wq_sb = singles.tile([P, KC, CW], bf16)
            nc.sync.dma_start(wq_sb, wqT.rearrange("(kc p) c -> p kc c", p=P))
            # q strip tiles ride a 2-slot ring; allocated at DMA-emission
            # time, closures resolve them through this dict at pop time
            qin = {}

            def load_q(t):
                qin[t] = qio.tile([P, KC, NT], bf16, tag="qin", name=f"qin{t}")
                nc.sync.dma_start(
                    qin[t], qT[:, ts(t, NT)].rearrange("(kc p) l -> p kc l", p=P))

            load_q(0)
            wk_sb = singles.tile([P, KC, CW], bf16)
            nc.sync.dma_start(wk_sb, wkT.rearrange("(kc p) c -> p kc c", p=P))
            kin = []
            for t in range(KTN):
                kin_t = singles.tile([P, KC, NT], bf16, name=f"kin{t}")
                nc.sync.dma_start(
                    kin_t, kT[:, ts(t, NT)].rearrange("(kc p) l -> p kc l", p=P))
                kin.append(kin_t)
            load_q(1)
            wo_sb = singles.tile([P, G, DIM], bf16)
            nc.sync.dma_start(wo_sb, woT.rearrange("(g p) j -> p g j", p=P))

            qhT = singles.tile([P, G, L], bf16)
            khT = singles.tile([P, G, M], bf16)
            khp = singles.tile([P, MG, HPC, CH], bf16)
            xu = singles.tile([P, G, L], bf16)
            ident = singles.tile([P, P], bf16)
            make_identity(nc, ident)

            ones_sb = singles.tile([P, 1], f32)
            nc.vector.memset(ones_sb, 1.0)
            for mg in range(MG):
                nc.vector.tensor_copy(khp[:, mg, :, HD:CH],
                                      ones_sb[:, None, :].to_broadcast([P, HPC, 1]))

            # ---- filler units (each <= ~2 matmuls of PE work) ----
            def proj_units(dst, w_sb, src, tt, g):
                """q/k projection of one (strip, group): 4 units x 2 MMs.
                src is a thunk resolved at pop time (q tiles alloc late)."""
                st = {}

                def unit(k, st=st):
                    if k == 0:
                        st["ps"] = psA.tile([P, NT], f32, tag="aux", name="pps")
                    ps = st["ps"]
                    src_t = src()
                    for kc in (2 * k, 2 * k + 1):
                        nc.tensor.matmul(ps, lhsT=w_sb[:, kc, ts(g, P)],
                                         rhs=src_t[:, kc],
                                         start=(kc == 0), stop=(kc == KC - 1))
                    if k == 3:
                        nc.vector.tensor_copy(dst[:, g, ts(tt, NT)], ps)

                return [lambda k=k: unit(k) for k in range(4)]

            def ktrans_unit(mc, g):
                def unit():
                    tr = psA.tile([P, P], bf16, tag="aux", name="trp")
                    nc.tensor.transpose(tr, khT[:, g, ts(mc, P)], ident)
                    for hh in range(2):
                        nc.vector.tensor_copy(khp[:, mc, g * 2 + hh, 0:HD],
                                              tr[:, ts(hh, HD)])
                return [unit]

            def po_unit(lc, jt, eng=None):
                def unit():
                    po = psA.tile([P, 512], f32, tag="aux", name="pop")
                    for cc in range(G):
                        nc.tensor.matmul(po, lhsT=xu[:, cc, ts(lc, P)],
                                         rhs=wo_sb[:, cc, ts(jt, 512)],
                                         start=(cc == 0), stop=(cc == G - 1))
                    ot = ost.tile([P, 512], f32, tag="ot")
                    nc.vector.tensor_copy(ot, po)
                    # stores ride sync by default, keeping gpsimd free for
                    # the latency-sensitive normalization round trips
                    (eng or nc.sync).dma_start(out[ts(lc, P), ts(jt, 512)], ot)
                return [unit]

            def q_src(t):
                return lambda: qin[t]

            def k_src(t):
                return lambda: kin[t]

            # ---- head phase: projections feeding sub-strip (0, g0) ----
            # Warm-up matmuls on the first-arriving weight tile: ~3.4us of
            # sustained PE activity flips the HAM clock gate to 2.4GHz before
            # the real projections run (cold MMs cost ~1.8x).
            def warmup(n):
                wps = psS.tile([P, 2 * NT], f32, tag="s", name="warm")
                for i in range(n):
                    nc.tensor.matmul(wps[:, 0:CW], lhsT=wq_sb[:, 0, 0:P],
                                     rhs=wq_sb[:, 0, :], start=True, stop=True)

            warmup(16)
            for g in range(G):
                for u in proj_units(qhT, wq_sb, q_src(0), 0, g):
                    u()
            warmup(12)
            for u in proj_units(khT, wk_sb, k_src(0), 0, 0):
                u()
            for mc in range(4):
                for u in ktrans_unit(mc, 0):
                    u()

            # ---- per-sub-strip filler queues ----
            def kproj_strip_fill(g, qp_after):
                """kproj m-tiles 1..3 (+m0 of g1) & ktrans, deadline-ordered."""
                fs = []
                fs += proj_units(khT, wk_sb, k_src(1), 1, g)
                fs += ktrans_unit(4, g) + ktrans_unit(5, g)
                fs += proj_units(khT, wk_sb, k_src(2), 2, g)
                for mc in range(6, 10):
                    fs += ktrans_unit(mc, g)
                fs += proj_units(khT, wk_sb, k_src(3), 3, g)
                for mc in range(10, MG):
                    fs += ktrans_unit(mc, g)
                fs += qp_after
                return deque(fs)

            fill = {}
            fill[(0, 0)] = kproj_strip_fill(
                0, proj_units(khT, wk_sb, k_src(0), 0, 1) +
                [u for mc in range(4) for u in ktrans_unit(mc, 1)])
            fill[(0, 1)] = kproj_strip_fill(1, proj_units(qhT, wq_sb, q_src(1), 1, 0))
            for l5 in range(1, L5):
                fs = deque(proj_units(qhT, wq_sb, q_src(l5), l5, 1))
                fs += [lambda: None, lambda: None]  # let prior strip's xu land
                for lc in range(4 * (l5 - 1), 4 * l5):
                    for jt in range(JT):
                        fs += po_unit(lc, jt)
                fill[(l5, 0)] = fs
                if l5 < L5 - 1:
                    fill[(l5, 1)] = deque(
                        proj_units(qhT, wq_sb, q_src(l5 + 1), l5 + 1, 0))
                else:
                    fill[(l5, 1)] = deque()
            budget = {k: (2 if k[0] == 0 else 1) for k in fill}

            # ---- main loop: one flat pipeline over all 8 sub-strips; the
            # 2-ahead score prefetch crosses sub-strip boundaries so ACT
            # never waits at a seam ----
            subs = [(l5, g) for l5 in range(L5) for g in range(G)]
            NP = len(subs) * MG

            def emit_sp_at(p):
                (l5, g), mc = subs[p // MG], p % MG
                lsl = ts(l5, NT)
                sps = psS.tile([P, 2 * NT], f32, tag="s", name="sps")
                nc.tensor.matmul(sps[:, 0:NT],
                                 lhsT=khT[0:HD, g, ts(mc, P)],
                                 rhs=qhT[0:HD, g, lsl],
                                 start=True, stop=True)
                nc.tensor.matmul(sps[:, NT:2 * NT],
                                 lhsT=khT[HD:P, g, ts(mc, P)],
                                 rhs=qhT[HD:P, g, lsl],
                                 start=True, stop=True)
                return sps

            sq = [emit_sp_at(0), emit_sp_at(1)]
            xpsA = xpsB = fq = nb = None
            for p in range(NP):
                (l5, g), mc = subs[p // MG], p % MG
                lsl = ts(l5, NT)
                if mc == 0:
                    if g == 0 and 1 <= l5 < L5 - 1:
                        load_q(l5 + 1)
                    xpsA = psX.tile([CH, NT], f32, tag="x", name="xpsA")
                    xpsB = psX.tile([CH, NT], f32, tag="x", name="xpsB")
                    fq, nb = fill[(l5, g)], budget[(l5, g)]
                for _ in range(min(nb, len(fq))):
                    fq.popleft()()
                if p + 2 < NP:
                    sq.append(emit_sp_at(p + 2))
                es = es_pool.tile([P, 2 * NT], bf16, tag="es")
                nc.scalar.activation(es, sq.pop(0), Exp, scale=0.125)
                nc.tensor.matmul(xpsA, lhsT=khp[:, mc, 2 * g, :],
                                 rhs=es[:, 0:NT],
                                 start=(mc == 0), stop=(mc == MG - 1))
                nc.tensor.matmul(xpsB, lhsT=khp[:, mc, 2 * g + 1, :],
                                 rhs=es[:, NT:2 * NT],
                                 start=(mc == 0), stop=(mc == MG - 1))
                if mc == MG - 1:
                    while fq:
                        fq.popleft()()
                    # drain X accumulators fast (frees PSUM), then normalize:
                    # rden = 1/row64 ; xu = X * rden  (DRAM bcast round trip)
                    xrs = []
                    for xps in (xpsA, xpsB):
                        xr = nrm.tile([CH, NT], f32, tag="xr", name="xr")
                        nc.vector.tensor_copy(xr, xps)
                        xrs.append(xr)
                    for hh, xr in enumerate(xrs):
                        # reciprocal runs wide ([64,NT], not [1,NT]: 6x faster)
                        h = 2 * g + hh
                        nc.gpsimd.dma_start(rd_dram[h:h + 1, lsl], xr[HD:CH])
                        dbc = nrm.tile([HD, NT], f32, tag="dbc", name="dbc")
                        nc.gpsimd.dma_start(
                            dbc, rd_dram[h:h + 1, lsl].to_broadcast([HD, NT]))
                        rdbc = nrm.tile([HD, NT], f32, tag="rdbc", name="rdbc")
                        nc.vector.reciprocal_approx_fast(rdbc, dbc)
                        nc.vector.tensor_mul(xu[ts(hh, HD), g, lsl],
                                             xr[0:HD], rdbc)

            # ---- tail: out-projection of the last strip; stores split
            # across both DMA queues to halve the final store drain ----
            for i, (lc, jt) in enumerate(
                    (lc, jt) for lc in range(4 * (L5 - 1), LC) for jt in range(JT)):
                for u in po_unit(lc, jt, nc.sync if i % 2 == 0 else nc.gpsimd):
                    u()

    nc.finalize()
    return nc


def _get_nc(L, M):
    key = (L, M)
    if key not in _cache:
        _cache[key] = _build(L, M)
    return _cache[key]


# head-major channel permutation: new channel c = h*64+d <- original column d*16+h
_PERM = np.array([(c % HD) * NH + c // HD for c in range(DIM)])

last_exec_time_ns = None
last_results = None


def kernel(q, k, v, Wq, Wk, Wv, Wo):  # noqa: ARG001 - v/Wv dead in reference
    global last_exec_time_ns, last_results
    q = np.asarray(q, np.float32)
    k = np.asarray(k, np.float32)
    Wq = np.asarray(Wq, np.float32)
    Wk = np.asarray(Wk, np.float32)
    Wo = np.asarray(Wo, np.float32)
    B, L, _ = q.shape
    M = k.shape[1]

    import ml_dtypes
    bf = ml_dtypes.bfloat16
    Wq_p = Wq[_PERM]            # (1024, 1024) head-major rows
    Wk_p = Wk[_PERM]
    WoT_p = Wo[:, _PERM].T      # (1024 c, 1024 j)

    qT = [np.ascontiguousarray(q[b].T).astype(bf) for b in range(B)]
    kT = [np.ascontiguousarray(k[b].T).astype(bf) for b in range(B)]
    wqT = [np.ascontiguousarray(Wq_p[hg * CW:(hg + 1) * CW, :].T).astype(bf) for hg in range(4)]
    wkT = [np.ascontiguousarray(Wk_p[hg * CW:(hg + 1) * CW, :].T).astype(bf) for hg in range(4)]
    woT = [np.ascontiguousarray(WoT_p[hg * CW:(hg + 1) * CW, :]).astype(bf) for hg in range(4)]

    in_maps = []
    for core in range(8):
        b, hg = divmod(core, 4)
        in_maps.append({"qT": qT[b], "kT": kT[b], "wqT": wqT[hg],
                        "wkT": wkT[hg], "woT": woT[hg]})

    nc = _get_nc(L, M)
    trace = bool(int(os.environ.get("MHA_TRACE", "0")))
    res = run_bass_kernel_spmd(nc, in_maps, core_ids=list(range(8)), trace=trace)
    last_results = res
    last_exec_time_ns = res.exec_time_ns

    out = np.zeros((B, L, DIM), np.float32)
    for core in range(8):
        b = core // 4
        out[b] += res.results[core]["out"]
    return out


# revision 35
# speedup vs baseline: 1.2762x; 1.0008x over previous
"""Trainium2 Bass kernel for nn_MultiHeadAttention_60559038873660.

Reference math (faithful to the source bug: attention is contracted with the
projected K, not V, so v/Wv are dead inputs):
    qp = q @ Wq.T ; kp = k @ Wk.T
    head split via reshape(b, l, 64, 16): head n takes strided columns {d*16+n}
    S = Qh @ Kh.T / 8 ; A = softmax(S, axis=m) ; X = A @ Kh ; out = X @ Wo.T

Strategy (v2 — strip-major, fully pipelined):
  - Host-side: permute weight rows/cols head-major so each head is a contiguous
    64-column block; pre-transpose q/k/weights into the layouts the TensorE
    wants (contraction on partitions).
  - 8 cores = 2 batches x 4 head-groups (4 heads each).  Each core computes its
    4 heads' attention plus a partial output projection; the host sums the 4
    partials per batch (tensor-parallel row-split reduction).
  - The per-core schedule is ScalarE-bound (softmax exp = 16.8M elem at
    1 elem/lane/cycle @1.2GHz ~ 147us).  Everything else hides under it:
      * strip-major order (l-strip outer, head-pair group inner) so the output
        projection + stores stream per strip instead of piling up at the end
      * attention starts ~11us in, after only {Wq, q-strip0, Wk, k-tile0} DMAs
        and the group-0 projections of those tiles; all remaining projection /
        transpose / out-projection work is pumped into the attention loop as
        fine-grained fillers between iterations (PE has ~40% slack vs ACT)
      * softmax denominators ride the X^T matmul as a fused ones-column (row
        64); normalization = DVE reciprocal + SBUF->SBUF DMA partition
        broadcast + fused multiply during the PSUM->SBUF drain (no DRAM trip)
      * PSUM: 4 banks score double-buffer, 2 banks X accum, 2 banks shared
        aux ring (projection / out-projection / transpose targets)
"""

import contextlib
import ctypes
import os
import sys
import types
from collections import deque

import numpy as np

import concourse.bacc as bacc
import concourse.tile as tile
from concourse import mybir
from concourse.bass import ds, ts
from concourse.bass_utils import run_bass_kernel_spmd


def _install_ntff_hook():
    """Provide antenv.axon_hooks if the image lacks it, wiring NTFF
    profiling straight into libaxon_pjrt.so (same ABI trn_boot uses)."""
    try:
        import antenv.axon_hooks  # noqa: F401
        return
    except ImportError:
        pass
    mod = types.ModuleType("antenv.axon_hooks")
    holder = [None]
    mod.set_axon_ntff_profile_hook = lambda h: holder.__setitem__(0, h)
    mod.get_axon_ntff_profile_hook = lambda: holder[0]
    sys.modules["antenv.axon_hooks"] = mod
    try:
        import antenv
        antenv.axon_hooks = mod
    except ImportError:
        pass

    so_path = "/opt/axon/libaxon_pjrt.so"
    if not os.path.exists(so_path):
        return
    lib = ctypes.CDLL(so_path)
    if not hasattr(lib, "axon_start_nrt_profile"):
        return
    lib.axon_start_nrt_profile.argtypes = [ctypes.POINTER(ctypes.c_int64), ctypes.c_size_t]
    lib.axon_start_nrt_profile.restype = ctypes.c_int64
    lib.axon_stop_nrt_profile.argtypes = [ctypes.c_char_p]
    lib.axon_stop_nrt_profile.restype = ctypes.c_int64

    @contextlib.contextmanager
    def _hook(output_dir, device_ids):
        import jax
        jax.devices()
        if device_ids:
            ids = (ctypes.c_int64 * len(device_ids))(*device_ids)
            rc = lib.axon_start_nrt_profile(ids, len(device_ids))
        else:
            rc = lib.axon_start_nrt_profile(None, 0)
        if rc != 0:
            raise RuntimeError(f"axon_start_nrt_profile rc={rc}")
        try:
            yield
        finally:
            n = lib.axon_stop_nrt_profile(str(output_dir).encode())
            print(f"profile: {n} file(s) written to {output_dir}", file=sys.stderr)

    mod.set_axon_ntff_profile_hook(_hook)


_install_ntff_hook()

f32 = mybir.dt.float32
f32r = mybir.dt.float32r
bf16 = mybir.dt.bfloat16
Exp = mybir.ActivationFunctionType.Exp

P = 128
DIM = 1024
NH = 16
HD = 64
HPC = 4          # heads per core
CW = HPC * HD    # 256 channel columns per core
CH = HD + 1      # head channels + ones column
G = CW // P      # 2 channel groups of 128
KC = DIM // P    # 8 contraction chunks for projections
JT = DIM // 512  # out-projection j tiles

_cache = {}


NTC = 512                    # input DMA tile width (host packing contract)


def _build(L, M):
    NT = 512                 # l-strip width / matmul moving tile
    L5 = L // NT             # 4 l-strips
    KTN = M // NT            # 4 k DMA tiles
    MG = M // P              # 16 m chunks per strip
    LC = L // P              # 16 out-proj l chunks

    nc = bacc.Bacc()
    # all inputs arrive pre-packed by the host into SBUF-tile layouts so
    # every load is contiguous 4-8KB rows per partition (full HBM bandwidth)
    qT = nc.declare_dram_parameter("qT", [L // NTC, P, KC, NTC], bf16, isOutput=False)
    kT = nc.declare_dram_parameter("kT", [M // NTC, P, KC, NTC], bf16, isOutput=False)
    wqT = nc.declare_dram_parameter("wqT", [P, KC, CW], bf16, isOutput=False)
    wkT = nc.declare_dram_parameter("wkT", [P, KC, CW], bf16, isOutput=False)
    woT = nc.declare_dram_parameter("woT", [P, G, DIM], bf16, isOutput=False)
    out = nc.declare_dram_parameter("out", [L, DIM], f32, isOutput=True)
    rd_dram = nc.dram_tensor("rden_scratch", [HPC, L], f32)

    from concourse.masks import make_identity

    with tile.TileContext(nc) as tc:
        with (
            tc.tile_pool(name="singles", bufs=1) as singles,
            tc.tile_pool(name="qio", bufs=2) as qio,
            tc.tile_pool(name="es", bufs=4) as es_pool,
            tc.tile_pool(name="nrm", bufs=2) as nrm,
            tc.tile_pool(name="ost", bufs=3) as ost,
            tc.tile_pool(name="psS", bufs=2, space="PSUM") as psS,
            tc.tile_pool(name="psX", bufs=2, space="PSUM") as psX,
            tc.tile_pool(name="psA", bufs=2, space="PSUM") as psA,
        ):
            # ---- input DMAs: minimal prefix first (wq, q0, wk, k0..3) ----
            wq_sb = singles.tile([P, KC, CW], bf16)
            nc.sync.dma_start(wq_sb, wqT[:])
            # q strip tiles ride a 2-slot ring; allocated at DMA-emission
            # time, closures resolve them through this dict at pop time
            qin = {}

            def load_q(t):
                qin[t] = qio.tile([P, KC, NT], bf16, tag="qin", name=f"qin{t}")
                nc.sync.dma_start(qin[t], qT[t])

            load_q(0)
            wk_sb = singles.tile([P, KC, CW], bf16)
            nc.sync.dma_start(wk_sb, wkT[:])
            kin = []
            for t in range(KTN):
                kin_t = singles.tile([P, KC, NT], bf16, name=f"kin{t}")
                nc.sync.dma_start(kin_t, kT[t])
                kin.append(kin_t)
            load_q(1)
            wo_sb = singles.tile([P, G, DIM], bf16)
            nc.sync.dma_start(wo_sb, woT[:])

            qhT = singles.tile([P, G, L], bf16)
            khT = singles.tile([P, G, M], bf16)
            khp = singles.tile([P, MG, HPC, CH], bf16)
            xu = singles.tile([P, G, L], bf16)
            ident = singles.tile([P, P], bf16)
            make_identity(nc, ident)

            ones_sb = singles.tile([P, 1], f32)
            nc.vector.memset(ones_sb, 1.0)
            ones_row = singles.tile([1, HD], bf16)
            nc.vector.memset(ones_row, 1.0)
            for mg in range(MG):
                nc.vector.tensor_copy(khp[:, mg, :, HD:CH],
                                      ones_sb[:, None, :].to_broadcast([P, HPC, 1]))

            # ---- filler units (each <= ~2 matmuls of PE work) ----
            def proj_units(dst, w_sb, src, tt, g):
                """q/k projection of one (strip, group): 4 units x 2 MMs.
                src is a thunk resolved at pop time (q tiles alloc late)."""
                st = {}

                def unit(k, st=st):
                    if k == 0:
                        st["ps"] = psA.tile([P, NT], f32, tag="aux", name="pps")
                    ps = st["ps"]
                    src_t = src()
                    for kc in (2 * k, 2 * k + 1):
                        nc.tensor.matmul(ps, lhsT=w_sb[:, kc, ts(g, P)],
                                         rhs=src_t[:, kc],
                                         start=(kc == 0), stop=(kc == KC - 1))
                    if k == 3:
                        nc.vector.tensor_copy(dst[:, g, ts(tt, NT)], ps)

                return [lambda k=k: unit(k) for k in range(4)]

            def ktrans_unit(mc, g):
                def unit():
                    tr = psA.tile([P, P], bf16, tag="aux", name="trp")
                    nc.tensor.transpose(tr, khT[:, g, ts(mc, P)], ident)
                    for hh in range(2):
                        nc.vector.tensor_copy(khp[:, mc, g * 2 + hh, 0:HD],
                                              tr[:, ts(hh, HD)])
                return [unit]

            def po_unit(lc, jt, eng=None):
                def unit():
                    po = psA.tile([P, 512], f32, tag="aux", name="pop")
                    for cc in range(G):
                        nc.tensor.matmul(po, lhsT=xu[:, cc, ts(lc, P)],
                                         rhs=wo_sb[:, cc, ts(jt, 512)],
                                         start=(cc == 0), stop=(cc == G - 1))
                    ot = ost.tile([P, 512], f32, tag="ot")
                    nc.vector.tensor_copy(ot, po)
                    # stores ride sync by default, keeping gpsimd free for
                    # the latency-sensitive normalization round trips
                    (eng or nc.sync).dma_start(out[ts(lc, P), ts(jt, 512)], ot)
                return [unit]

            def q_src(t):
                return lambda: qin[t]

            def k_src(t):
                return lambda: kin[t]

            def kproj_chunk_unit(c, g):
                """narrow (N=128) k projection of one m-chunk -> khT."""
                def unit():
                    ps = psA.tile([P, P], f32, tag="aux", name="kcp")
                    for kc in range(KC):
                        nc.tensor.matmul(ps, lhsT=wk_sb[:, kc, ts(g, P)],
                                         rhs=kin[c // 4][:, kc, ts(c % 4, P)],
                                         start=(kc == 0), stop=(kc == KC - 1))
                    nc.vector.tensor_copy(khT[:, g, ts(c, P)], ps)
                return [unit]

            # ---- head phase: minimal prefix feeding sub-strip (0, g0) ----
            # Warm-up matmuls on the first-arriving weight tile: ~3.4us of
            # sustained PE activity flips the HAM clock gate to 2.4GHz before
            # the real projections run (cold MMs cost ~1.8x).
            def warmup(n):
                wps = psS.tile([P, 2 * NT], f32, tag="s", name="warm")
                for i in range(n):
                    nc.tensor.matmul(wps[:, 0:CW], lhsT=wq_sb[:, 0, 0:P],
                                     rhs=wq_sb[:, 0, :], start=True, stop=True)

            warmup(10)
            for g in range(G):
                for u in proj_units(qhT, wq_sb, q_src(0), 0, g):
                    u()
            warmup(4)
            for u in kproj_chunk_unit(0, 0):
                u()

            # ---- per-sub-strip filler queues ----
            def kproj_strip_fill(g, qp_after):
                """kproj m-tiles 1..3 (+m0 of g1) & ktrans, deadline-ordered."""
                fs = []
                fs += proj_units(khT, wk_sb, k_src(1), 1, g)
                fs += ktrans_unit(4, g) + ktrans_unit(5, g)
                fs += proj_units(khT, wk_sb, k_src(2), 2, g)
                for mc in range(6, 10):
                    fs += ktrans_unit(mc, g)
                fs += proj_units(khT, wk_sb, k_src(3), 3, g)
                for mc in range(10, MG):
                    fs += ktrans_unit(mc, g)
                fs += qp_after
                return deque(fs)

            fill = {}
            # (0,g0) finishes kproj g0 m-chunks 1-3 narrow (sp deadlines are
            # per-chunk at the pipeline head), then the wide m-tiles
            f00 = kproj_chunk_unit(1, 0) + kproj_chunk_unit(2, 0) \
                + ktrans_unit(0, 0) + kproj_chunk_unit(3, 0) \
                + ktrans_unit(1, 0) + ktrans_unit(2, 0) \
                + proj_units(khT, wk_sb, k_src(1), 1, 0) \
                + ktrans_unit(3, 0) + ktrans_unit(4, 0) + ktrans_unit(5, 0) \
                + proj_units(khT, wk_sb, k_src(2), 2, 0)
            for mc in range(6, 10):
                f00 += ktrans_unit(mc, 0)
            f00 += proj_units(khT, wk_sb, k_src(3), 3, 0)
            for mc in range(10, MG):
                f00 += ktrans_unit(mc, 0)
            f00 += proj_units(khT, wk_sb, k_src(0), 0, 1)
            for mc in range(4):
                f00 += ktrans_unit(mc, 1)
            fill[(0, 0)] = deque(f00)
            fill[(0, 1)] = kproj_strip_fill(1, proj_units(qhT, wq_sb, q_src(1), 1, 0))
            for l5 in range(1, L5):
                fs = deque(proj_units(qhT, wq_sb, q_src(l5), l5, 1))
                fs += [lambda: None, lambda: None]  # let prior strip's xu land
                for lc in range(4 * (l5 - 1), 4 * l5):
                    for jt in range(JT):
                        fs += po_unit(lc, jt)
                fill[(l5, 0)] = fs
                if l5 < L5 - 1:
                    fill[(l5, 1)] = deque(
                        proj_units(qhT, wq_sb, q_src(l5 + 1), l5 + 1, 0))
                else:
                    fill[(l5, 1)] = deque()
            budget = {k: 1 for k in fill}
            budget[(0, 0)] = 4
            budget[(0, 1)] = 2

            # ---- main loop: one flat pipeline over all 8 sub-strips; the
            # 2-ahead score prefetch crosses sub-strip boundaries so ACT
            # never waits at a seam ----
            subs = [(l5, g) for l5 in range(L5) for g in range(G)]
            NP = len(subs) * MG

            def emit_sp_at(p):
                (l5, g), mc = subs[p // MG], p % MG
                lsl = ts(l5, NT)
                sps = psS.tile([P, 2 * NT], f32, tag="s", name="sps")
                nc.tensor.matmul(sps[:, 0:NT],
                                 lhsT=khT[0:HD, g, ts(mc, P)],
                                 rhs=qhT[0:HD, g, lsl],
                                 start=True, stop=True)
                nc.tensor.matmul(sps[:, NT:2 * NT],
                                 lhsT=khT[HD:P, g, ts(mc, P)],
                                 rhs=qhT[HD:P, g, lsl],
                                 start=True, stop=True)
                return sps

            sq = [emit_sp_at(0)]
            xpsA = xpsB = fq = nb = None
            for p in range(NP):
                (l5, g), mc = subs[p // MG], p % MG
                lsl = ts(l5, NT)
                if mc == 0:
                    if g == 0 and 1 <= l5 < L5 - 1:
                        load_q(l5 + 1)
                    xpsA = psX.tile([CH, NT], f32, tag="x", name="xpsA")
                    xpsB = psX.tile([CH, NT], f32, tag="x", name="xpsB")
                    fq, nb = fill[(l5, g)], budget[(l5, g)]
                for _ in range(min(nb, len(fq))):
                    fq.popleft()()
                if p == 0:
                    # sp(1) waits here: its khT chunk is projected by the
                    # first fillers popped above
                    sq.append(emit_sp_at(1))
                if p + 2 < NP:
                    sq.append(emit_sp_at(p + 2))
                es = es_pool.tile([P, 2 * NT], bf16, tag="es")
                nc.scalar.activation(es, sq.pop(0), Exp, scale=0.125)
                nc.tensor.matmul(xpsA, lhsT=khp[:, mc, 2 * g, :],
                                 rhs=es[:, 0:NT],
                                 start=(mc == 0), stop=(mc == MG - 1))
                nc.tensor.matmul(xpsB, lhsT=khp[:, mc, 2 * g + 1, :],
                                 rhs=es[:, NT:2 * NT],
                                 start=(mc == 0), stop=(mc == MG - 1))
                if mc == MG - 1:
                    while fq:
                        fq.popleft()()
                    # drain X accumulators fast (frees PSUM), then normalize
                    # xu = X / row64
                    xrs = []
                    for xps in (xpsA, xpsB):
                        xr = nrm.tile([CH, NT], f32, tag="xr", name="xr")
                        nc.vector.tensor_copy(xr, xps)
                        xrs.append(xr)
                    for hh, xr in enumerate(xrs):
                        if p == NP - 1 and not os.environ.get("MHA_NO_BC"):
                            # final sub-strip: no DMA-latency budget left.
                            # Broadcast 1/den across partitions with a K=1
                            # PE matmul into the now-idle score PSUM ring.
                            rden = nrm.tile([1, NT], f32, tag="rdn", name="rdn")
                            nc.vector.reciprocal_approx_fast(rden, xr[HD:CH])
                            rdenb = nrm.tile([1, NT], bf16, tag="rdnb", name="rdnb")
                            nc.vector.tensor_copy(rdenb, rden)
                            bc = psS.tile([P, 2 * NT], f32, tag="s", name="bc")
                            nc.tensor.matmul(
                                bc[0:HD, 0:NT],
                                lhsT=ones_row, rhs=rdenb,
                                start=True, stop=True)
                            nc.vector.tensor_mul(xu[ts(hh, HD), g, lsl],
                                                 xr[0:HD], bc[0:HD, 0:NT])
                            continue
                        # mid-stream: DRAM round-trip broadcast (latency
                        # hides under the next sub-strip); reciprocal runs
                        # wide ([64,NT], not [1,NT]: 6x faster)
                        h = 2 * g + hh
                        nc.gpsimd.dma_start(rd_dram[h:h + 1, lsl], xr[HD:CH])
                        dbc = nrm.tile([HD, NT], f32, tag="dbc", name="dbc")
                        nc.gpsimd.dma_start(
                            dbc, rd_dram[h:h + 1, lsl].to_broadcast([HD, NT]))
                        rdbc = nrm.tile([HD, NT], f32, tag="rdbc", name="rdbc")
                        nc.vector.reciprocal_approx_fast(rdbc, dbc)
                        nc.vector.tensor_mul(xu[ts(hh, HD), g, lsl],
                                             xr[0:HD], rdbc)

            # ---- tail: out-projection of the last strip; stores split
            # across both DMA queues to halve the final store drain ----
            for i, (lc, jt) in enumerate(
                    (lc, jt) for lc in range(4 * (L5 - 1), LC) for jt in range(JT)):
                for u in po_unit(lc, jt, nc.sync if i % 2 == 0 else nc.gpsimd):
                    u()

    nc.finalize()
    return nc


def _get_nc(L, M):
    key = (L, M)
    if key not in _cache:
        _cache[key] = _build(L, M)
    return _cache[key]


# head-major channel permutation: new channel c = h*64+d <- original column d*16+h
_PERM = np.array([(c % HD) * NH + c // HD for c in range(DIM)])

last_exec_time_ns = None
last_results = None


def kernel(q, k, v, Wq, Wk, Wv, Wo):  # noqa: ARG001 - v/Wv dead in reference
    global last_exec_time_ns, last_results
    q = np.asarray(q, np.float32)
    k = np.asarray(k, np.float32)
    Wq = np.asarray(Wq, np.float32)
    Wk = np.asarray(Wk, np.float32)
    Wo = np.asarray(Wo, np.float32)
    B, L, _ = q.shape
    M = k.shape[1]

    import ml_dtypes
    bf = ml_dtypes.bfloat16
    Wq_p = Wq[_PERM]            # (1024, 1024) head-major rows
    Wk_p = Wk[_PERM]
    WoT_p = Wo[:, _PERM].T      # (1024 c, 1024 j)

    # pack into the kernel's SBUF-tile layouts so every DMA row is
    # contiguous (strided loads run at ~1/3 of HBM bandwidth)
    def pack_in(xT):            # [DIM, L] -> [L/NTC, P, KC, NTC]
        return np.ascontiguousarray(
            xT.reshape(KC, P, -1, NTC).transpose(2, 1, 0, 3)).astype(bf)

    def pack_w(wT):             # [DIM, CW] -> [P, KC, CW]
        return np.ascontiguousarray(
            wT.reshape(KC, P, CW).transpose(1, 0, 2)).astype(bf)

    qT = [pack_in(q[b].T) for b in range(B)]
    kT = [pack_in(k[b].T) for b in range(B)]
    wqT = [pack_w(Wq_p[hg * CW:(hg + 1) * CW, :].T) for hg in range(4)]
    wkT = [pack_w(Wk_p[hg * CW:(hg + 1) * CW, :].T) for hg in range(4)]
    woT = [np.ascontiguousarray(
        WoT_p[hg * CW:(hg + 1) * CW, :].reshape(G, P, DIM).transpose(1, 0, 2)
    ).astype(bf) for hg in range(4)]

    in_maps = []
    for core in range(8):
        b, hg = divmod(core, 4)
        in_maps.append({"qT": qT[b], "kT": kT[b], "wqT": wqT[hg],
                        "wkT": wkT[hg], "woT": woT[hg]})

    nc = _get_nc(L, M)
    trace = bool(int(os.environ.get("MHA_TRACE", "0")))
    res = run_bass_kernel_spmd(nc, in_maps, core_ids=list(range(8)), trace=trace)
    last_results = res
    last_exec_time_ns = res.exec_time_ns

    out = np.zeros((B, L, DIM), np.float32)
    for core in range(8):
        b = core // 4
        out[b] += res.results[core]["out"]
    return out


# revision 36
# speedup vs baseline: 1.2923x; 1.0126x over previous
"""Trainium2 Bass kernel for nn_MultiHeadAttention_60559038873660.

Reference math (faithful to the source bug: attention is contracted with the
projected K, not V, so v/Wv are dead inputs):
    qp = q @ Wq.T ; kp = k @ Wk.T
    head split via reshape(b, l, 64, 16): head n takes strided columns {d*16+n}
    S = Qh @ Kh.T / 8 ; A = softmax(S, axis=m) ; X = A @ Kh ; out = X @ Wo.T

Strategy (v2 — strip-major, fully pipelined):
  - Host-side: permute weight rows/cols head-major so each head is a contiguous
    64-column block; pre-transpose q/k/weights into the layouts the TensorE
    wants (contraction on partitions).
  - 8 cores = 2 batches x 4 head-groups (4 heads each).  Each core computes its
    4 heads' attention plus a partial output projection; the host sums the 4
    partials per batch (tensor-parallel row-split reduction).
  - The per-core schedule is ScalarE-bound (softmax exp = 16.8M elem at
    1 elem/lane/cycle @1.2GHz ~ 147us).  Everything else hides under it:
      * strip-major order (l-strip outer, head-pair group inner) so the output
        projection + stores stream per strip instead of piling up at the end
      * attention starts ~11us in, after only {Wq, q-strip0, Wk, k-tile0} DMAs
        and the group-0 projections of those tiles; all remaining projection /
        transpose / out-projection work is pumped into the attention loop as
        fine-grained fillers between iterations (PE has ~40% slack vs ACT)
      * softmax denominators ride the X^T matmul as a fused ones-column (row
        64); normalization = DVE reciprocal + SBUF->SBUF DMA partition
        broadcast + fused multiply during the PSUM->SBUF drain (no DRAM trip)
      * PSUM: 4 banks score double-buffer, 2 banks X accum, 2 banks shared
        aux ring (projection / out-projection / transpose targets)
"""

import contextlib
import ctypes
import os
import sys
import types
from collections import deque

import numpy as np

import concourse.bacc as bacc
import concourse.tile as tile
from concourse import mybir
from concourse.bass import ds, ts
from concourse.bass_utils import run_bass_kernel_spmd


def _install_ntff_hook():
    """Provide antenv.axon_hooks if the image lacks it, wiring NTFF
    profiling straight into libaxon_pjrt.so (same ABI trn_boot uses)."""
    try:
        import antenv.axon_hooks  # noqa: F401
        return
    except ImportError:
        pass
    mod = types.ModuleType("antenv.axon_hooks")
    holder = [None]
    mod.set_axon_ntff_profile_hook = lambda h: holder.__setitem__(0, h)
    mod.get_axon_ntff_profile_hook = lambda: holder[0]
    sys.modules["antenv.axon_hooks"] = mod
    try:
        import antenv
        antenv.axon_hooks = mod
    except ImportError:
        pass

    so_path = "/opt/axon/libaxon_pjrt.so"
    if not os.path.exists(so_path):
        return
    lib = ctypes.CDLL(so_path)
    if not hasattr(lib, "axon_start_nrt_profile"):
        return
    lib.axon_start_nrt_profile.argtypes = [ctypes.POINTER(ctypes.c_int64), ctypes.c_size_t]
    lib.axon_start_nrt_profile.restype = ctypes.c_int64
    lib.axon_stop_nrt_profile.argtypes = [ctypes.c_char_p]
    lib.axon_stop_nrt_profile.restype = ctypes.c_int64

    @contextlib.contextmanager
    def _hook(output_dir, device_ids):
        import jax
        jax.devices()
        if device_ids:
            ids = (ctypes.c_int64 * len(device_ids))(*device_ids)
            rc = lib.axon_start_nrt_profile(ids, len(device_ids))
        else:
            rc = lib.axon_start_nrt_profile(None, 0)
        if rc != 0:
            raise RuntimeError(f"axon_start_nrt_profile rc={rc}")
        try:
            yield
        finally:
            n = lib.axon_stop_nrt_profile(str(output_dir).encode())
            print(f"profile: {n} file(s) written to {output_dir}", file=sys.stderr)

    mod.set_axon_ntff_profile_hook(_hook)


_install_ntff_hook()

f32 = mybir.dt.float32
f32r = mybir.dt.float32r
bf16 = mybir.dt.bfloat16
Exp = mybir.ActivationFunctionType.Exp

P = 128
DIM = 1024
NH = 16
HD = 64
HPC = 4          # heads per core
CW = HPC * HD    # 256 channel columns per core
CH = HD + 1      # head channels + ones column
G = CW // P      # 2 channel groups of 128
KC = DIM // P    # 8 contraction chunks for projections
JT = DIM // 512  # out-projection j tiles

_cache = {}


NTC = 512                    # input DMA tile width (host packing contract)


def _build(L, M):
    NT = 512                 # l-strip width / matmul moving tile
    L5 = L // NT             # 4 l-strips
    KTN = M // NT            # 4 k DMA tiles
    MG = M // P              # 16 m chunks per strip
    LC = L // P              # 16 out-proj l chunks

    nc = bacc.Bacc()
    # all inputs arrive pre-packed by the host into SBUF-tile layouts so
    # every load is contiguous 4-8KB rows per partition (full HBM bandwidth)
    qT = nc.declare_dram_parameter("qT", [L // NTC, P, KC, NTC], bf16, isOutput=False)
    kT = nc.declare_dram_parameter("kT", [M // NTC, P, KC, NTC], bf16, isOutput=False)
    wqT = nc.declare_dram_parameter("wqT", [P, KC, CW], bf16, isOutput=False)
    wkT = nc.declare_dram_parameter("wkT", [P, KC, CW], bf16, isOutput=False)
    woT = nc.declare_dram_parameter("woT", [P, G, DIM], bf16, isOutput=False)
    out = nc.declare_dram_parameter("out", [L, DIM], f32, isOutput=True)
    rd_dram = nc.dram_tensor("rden_scratch", [HPC, L], f32)

    from concourse.masks import make_identity

    with tile.TileContext(nc) as tc:
        with (
            tc.tile_pool(name="singles", bufs=1) as singles,
            tc.tile_pool(name="qio", bufs=2) as qio,
            tc.tile_pool(name="es", bufs=4) as es_pool,
            tc.tile_pool(name="nrm", bufs=2) as nrm,
            tc.tile_pool(name="ost", bufs=3) as ost,
            tc.tile_pool(name="psS", bufs=2, space="PSUM") as psS,
            tc.tile_pool(name="psX", bufs=2, space="PSUM") as psX,
            tc.tile_pool(name="psA", bufs=2, space="PSUM") as psA,
        ):
            # ---- input DMAs: minimal prefix first (wq, q0, wk, k0..3) ----
            wq_sb = singles.tile([P, KC, CW], bf16)
            nc.sync.dma_start(wq_sb, wqT[:])
            # q strip tiles ride a 2-slot ring; allocated at DMA-emission
            # time, closures resolve them through this dict at pop time
            qin = {}

            def load_q(t):
                qin[t] = qio.tile([P, KC, NT], bf16, tag="qin", name=f"qin{t}")
                nc.sync.dma_start(qin[t], qT[t])

            load_q(0)
            wk_sb = singles.tile([P, KC, CW], bf16)
            nc.sync.dma_start(wk_sb, wkT[:])
            kin = []
            for t in range(KTN):
                kin_t = singles.tile([P, KC, NT], bf16, name=f"kin{t}")
                nc.sync.dma_start(kin_t, kT[t])
                kin.append(kin_t)
            load_q(1)
            wo_sb = singles.tile([P, G, DIM], bf16)
            nc.sync.dma_start(wo_sb, woT[:])

            qhT = singles.tile([P, G, L], bf16)
            khT = singles.tile([P, G, M], bf16)
            khp = singles.tile([P, MG, HPC, CH], bf16)
            xu = singles.tile([P, G, L], bf16)
            ident = singles.tile([P, P], bf16)
            make_identity(nc, ident)

            ones_sb = singles.tile([P, 1], f32)
            nc.vector.memset(ones_sb, 1.0)
            ones_row = singles.tile([1, HD], bf16)
            nc.vector.memset(ones_row, 1.0)
            for mg in range(MG):
                nc.vector.tensor_copy(khp[:, mg, :, HD:CH],
                                      ones_sb[:, None, :].to_broadcast([P, HPC, 1]))

            # ---- filler units (each <= ~2 matmuls of PE work) ----
            def proj_units(dst, w_sb, src, tt, g):
                """q/k projection of one (strip, group): 4 units x 2 MMs.
                src is a thunk resolved at pop time (q tiles alloc late)."""
                st = {}

                def unit(k, st=st):
                    if k == 0:
                        st["ps"] = psA.tile([P, NT], f32, tag="aux", name="pps")
                    ps = st["ps"]
                    src_t = src()
                    for kc in (2 * k, 2 * k + 1):
                        nc.tensor.matmul(ps, lhsT=w_sb[:, kc, ts(g, P)],
                                         rhs=src_t[:, kc],
                                         start=(kc == 0), stop=(kc == KC - 1))
                    if k == 3:
                        nc.vector.tensor_copy(dst[:, g, ts(tt, NT)], ps)

                return [lambda k=k: unit(k) for k in range(4)]

            def ktrans_unit(mc, g):
                def unit():
                    tr = psA.tile([P, P], bf16, tag="aux", name="trp")
                    nc.tensor.transpose(tr, khT[:, g, ts(mc, P)], ident)
                    for hh in range(2):
                        nc.vector.tensor_copy(khp[:, mc, g * 2 + hh, 0:HD],
                                              tr[:, ts(hh, HD)])
                return [unit]

            def po_unit(lc, jt, eng=None):
                def unit():
                    po = psA.tile([P, 512], f32, tag="aux", name="pop")
                    for cc in range(G):
                        nc.tensor.matmul(po, lhsT=xu[:, cc, ts(lc, P)],
                                         rhs=wo_sb[:, cc, ts(jt, 512)],
                                         start=(cc == 0), stop=(cc == G - 1))
                    ot = ost.tile([P, 512], f32, tag="ot")
                    nc.vector.tensor_copy(ot, po)
                    # stores ride sync by default, keeping gpsimd free for
                    # the latency-sensitive normalization round trips
                    (eng or nc.sync).dma_start(out[ts(lc, P), ts(jt, 512)], ot)
                return [unit]

            def q_src(t):
                return lambda: qin[t]

            def k_src(t):
                return lambda: kin[t]

            def kproj_chunk_unit(c, g):
                """narrow (N=128) k projection of one m-chunk -> khT."""
                def unit():
                    ps = psA.tile([P, P], f32, tag="aux", name="kcp")
                    for kc in range(KC):
                        nc.tensor.matmul(ps, lhsT=wk_sb[:, kc, ts(g, P)],
                                         rhs=kin[c // 4][:, kc, ts(c % 4, P)],
                                         start=(kc == 0), stop=(kc == KC - 1))
                    nc.vector.tensor_copy(khT[:, g, ts(c, P)], ps)
                return [unit]

            # ---- head phase: minimal prefix feeding sub-strip (0, g0) ----
            # Warm-up matmuls on the first-arriving weight tile: ~3.4us of
            # sustained PE activity flips the HAM clock gate to 2.4GHz before
            # the real projections run (cold MMs cost ~1.8x).
            def warmup(n):
                wps = psS.tile([P, 2 * NT], f32, tag="s", name="warm")
                for i in range(n):
                    nc.tensor.matmul(wps[:, 0:CW], lhsT=wq_sb[:, 0, 0:P],
                                     rhs=wq_sb[:, 0, :], start=True, stop=True)

            warmup(10)
            for g in range(G):
                for u in proj_units(qhT, wq_sb, q_src(0), 0, g):
                    u()
            warmup(4)
            for u in kproj_chunk_unit(0, 0):
                u()

            # ---- per-sub-strip filler queues ----
            def kproj_strip_fill(g, qp_after):
                """kproj m-tiles 1..3 (+m0 of g1) & ktrans, deadline-ordered."""
                fs = []
                fs += proj_units(khT, wk_sb, k_src(1), 1, g)
                fs += ktrans_unit(4, g) + ktrans_unit(5, g)
                fs += proj_units(khT, wk_sb, k_src(2), 2, g)
                for mc in range(6, 10):
                    fs += ktrans_unit(mc, g)
                fs += proj_units(khT, wk_sb, k_src(3), 3, g)
                for mc in range(10, MG):
                    fs += ktrans_unit(mc, g)
                fs += qp_after
                return deque(fs)

            fill = {}
            # (0,g0) finishes kproj g0 m-chunks 1-3 narrow (sp deadlines are
            # per-chunk at the pipeline head), then the wide m-tiles
            f00 = kproj_chunk_unit(1, 0) + kproj_chunk_unit(2, 0) \
                + ktrans_unit(0, 0) + kproj_chunk_unit(3, 0) \
                + ktrans_unit(1, 0) + ktrans_unit(2, 0) \
                + proj_units(khT, wk_sb, k_src(1), 1, 0) \
                + ktrans_unit(3, 0) + ktrans_unit(4, 0) + ktrans_unit(5, 0) \
                + proj_units(khT, wk_sb, k_src(2), 2, 0)
            for mc in range(6, 10):
                f00 += ktrans_unit(mc, 0)
            f00 += proj_units(khT, wk_sb, k_src(3), 3, 0)
            for mc in range(10, MG):
                f00 += ktrans_unit(mc, 0)
            f00 += proj_units(khT, wk_sb, k_src(0), 0, 1)
            for mc in range(4):
                f00 += ktrans_unit(mc, 1)
            fill[(0, 0)] = deque(f00)
            fill[(0, 1)] = kproj_strip_fill(1, proj_units(qhT, wq_sb, q_src(1), 1, 0))
            for l5 in range(1, L5):
                fs = deque(proj_units(qhT, wq_sb, q_src(l5), l5, 1))
                fs += [lambda: None, lambda: None]  # let prior strip's xu land
                for lc in range(4 * (l5 - 1), 4 * l5):
                    for jt in range(JT):
                        fs += po_unit(lc, jt)
                fill[(l5, 0)] = fs
                if l5 < L5 - 1:
                    fill[(l5, 1)] = deque(
                        proj_units(qhT, wq_sb, q_src(l5 + 1), l5 + 1, 0))
                else:
                    fill[(l5, 1)] = deque()
            budget = {k: 1 for k in fill}
            budget[(0, 0)] = 4
            budget[(0, 1)] = 2

            # ---- main loop: one flat pipeline over all 8 sub-strips; the
            # 2-ahead score prefetch crosses sub-strip boundaries so ACT
            # never waits at a seam ----
            subs = [(l5, g) for l5 in range(L5) for g in range(G)]
            NP = len(subs) * MG

            def emit_sp_at(p):
                (l5, g), mc = subs[p // MG], p % MG
                lsl = ts(l5, NT)
                sps = psS.tile([P, 2 * NT], f32, tag="s", name="sps")
                nc.tensor.matmul(sps[:, 0:NT],
                                 lhsT=khT[0:HD, g, ts(mc, P)],
                                 rhs=qhT[0:HD, g, lsl],
                                 start=True, stop=True)
                nc.tensor.matmul(sps[:, NT:2 * NT],
                                 lhsT=khT[HD:P, g, ts(mc, P)],
                                 rhs=qhT[HD:P, g, lsl],
                                 start=True, stop=True)
                return sps

            sq = [emit_sp_at(0)]
            xpsA = xpsB = fq = nb = None
            for p in range(NP):
                (l5, g), mc = subs[p // MG], p % MG
                lsl = ts(l5, NT)
                if mc == 0:
                    if g == 0 and 1 <= l5 < L5 - 1:
                        load_q(l5 + 1)
                    xpsA = psX.tile([CH, NT], f32, tag="x", name="xpsA")
                    xpsB = psX.tile([CH, NT], f32, tag="x", name="xpsB")
                    fq, nb = fill[(l5, g)], budget[(l5, g)]
                for _ in range(min(nb, len(fq))):
                    fq.popleft()()
                if p == 0:
                    # sp(1) waits here: its khT chunk is projected by the
                    # first fillers popped above
                    sq.append(emit_sp_at(1))
                if p + 2 < NP:
                    sq.append(emit_sp_at(p + 2))
                es = es_pool.tile([P, 2 * NT], bf16, tag="es")
                nc.scalar.activation(es, sq.pop(0), Exp, scale=0.125)
                nc.tensor.matmul(xpsA, lhsT=khp[:, mc, 2 * g, :],
                                 rhs=es[:, 0:NT],
                                 start=(mc == 0), stop=(mc == MG - 1))
                nc.tensor.matmul(xpsB, lhsT=khp[:, mc, 2 * g + 1, :],
                                 rhs=es[:, NT:2 * NT],
                                 start=(mc == 0), stop=(mc == MG - 1))
                if mc == MG - 1:
                    while fq:
                        fq.popleft()()
                    # drain X accumulators fast (frees PSUM), then normalize
                    # xu = X / row64
                    xrs = []
                    for xps in (xpsA, xpsB):
                        xr = nrm.tile([CH, NT], f32, tag="xr", name="xr")
                        nc.vector.tensor_copy(xr, xps)
                        xrs.append(xr)
                    for hh, xr in enumerate(xrs):
                        if p == NP - 1 and not os.environ.get("MHA_NO_BC"):
                            # final sub-strip: no DMA-latency budget left.
                            # Broadcast 1/den across partitions with a K=1
                            # PE matmul into the now-idle score PSUM ring.
                            # (recip_approx NaNs on partition-offset inputs
                            # -> stage the den row to partition 0 first)
                            dn0 = nrm.tile([1, NT], f32, tag="dn0", name="dn0")
                            nc.vector.tensor_copy(dn0, xr[HD:CH])
                            rden = nrm.tile([1, NT], f32, tag="rdn", name="rdn")
                            nc.vector.reciprocal_approx_fast(rden, dn0)
                            rdenb = nrm.tile([1, NT], bf16, tag="rdnb", name="rdnb")
                            nc.vector.tensor_copy(rdenb, rden)
                            bc = psS.tile([P, 2 * NT], f32, tag="s", name="bc")
                            nc.tensor.matmul(
                                bc[0:HD, 0:NT],
                                lhsT=ones_row, rhs=rdenb,
                                start=True, stop=True)
                            nc.vector.tensor_mul(xu[ts(hh, HD), g, lsl],
                                                 xr[0:HD], bc[0:HD, 0:NT])
                            continue
                        # mid-stream: DRAM round-trip broadcast (latency
                        # hides under the next sub-strip); reciprocal runs
                        # wide ([64,NT], not [1,NT]: 6x faster)
                        h = 2 * g + hh
                        nc.gpsimd.dma_start(rd_dram[h:h + 1, lsl], xr[HD:CH])
                        dbc = nrm.tile([HD, NT], f32, tag="dbc", name="dbc")
                        nc.gpsimd.dma_start(
                            dbc, rd_dram[h:h + 1, lsl].to_broadcast([HD, NT]))
                        rdbc = nrm.tile([HD, NT], f32, tag="rdbc", name="rdbc")
                        nc.vector.reciprocal_approx_fast(rdbc, dbc)
                        nc.vector.tensor_mul(xu[ts(hh, HD), g, lsl],
                                             xr[0:HD], rdbc)

            # ---- tail: out-projection of the last strip; stores split
            # across both DMA queues to halve the final store drain ----
            for i, (lc, jt) in enumerate(
                    (lc, jt) for lc in range(4 * (L5 - 1), LC) for jt in range(JT)):
                for u in po_unit(lc, jt, nc.sync if i % 2 == 0 else nc.gpsimd):
                    u()

    nc.finalize()
    return nc


def _get_nc(L, M):
    key = (L, M)
    if key not in _cache:
        _cache[key] = _build(L, M)
    return _cache[key]


# head-major channel permutation: new channel c = h*64+d <- original column d*16+h
_PERM = np.array([(c % HD) * NH + c // HD for c in range(DIM)])

last_exec_time_ns = None
last_results = None


def kernel(q, k, v, Wq, Wk, Wv, Wo):  # noqa: ARG001 - v/Wv dead in reference
    global last_exec_time_ns, last_results
    q = np.asarray(q, np.float32)
    k = np.asarray(k, np.float32)
    Wq = np.asarray(Wq, np.float32)
    Wk = np.asarray(Wk, np.float32)
    Wo = np.asarray(Wo, np.float32)
    B, L, _ = q.shape
    M = k.shape[1]

    import ml_dtypes
    bf = ml_dtypes.bfloat16
    Wq_p = Wq[_PERM]            # (1024, 1024) head-major rows
    Wk_p = Wk[_PERM]
    WoT_p = Wo[:, _PERM].T      # (1024 c, 1024 j)

    # pack into the kernel's SBUF-tile layouts so every DMA row is
    # contiguous (strided loads run at ~1/3 of HBM bandwidth)
    def pack_in(xT):            # [DIM, L] -> [L/NTC, P, KC, NTC]
        return np.ascontiguousarray(
            xT.reshape(KC, P, -1, NTC).transpose(2, 1, 0, 3)).astype(bf)

    def pack_w(wT):             # [DIM, CW] -> [P, KC, CW]
        return np.ascontiguousarray(
            wT.reshape(KC, P, CW).transpose(1, 0, 2)).astype(bf)

    qT = [pack_in(q[b].T) for b in range(B)]
    kT = [pack_in(k[b].T) for b in range(B)]
    wqT = [pack_w(Wq_p[hg * CW:(hg + 1) * CW, :].T) for hg in range(4)]
    wkT = [pack_w(Wk_p[hg * CW:(hg + 1) * CW, :].T) for hg in range(4)]
    woT = [np.ascontiguousarray(
        WoT_p[hg * CW:(hg + 1) * CW, :].reshape(G, P, DIM).transpose(1, 0, 2)
    ).astype(bf) for hg in range(4)]

    in_maps = []
    for core in range(8):
        b, hg = divmod(core, 4)
        in_maps.append({"qT": qT[b], "kT": kT[b], "wqT": wqT[hg],
                        "wkT": wkT[hg], "woT": woT[hg]})

    nc = _get_nc(L, M)
    trace = bool(int(os.environ.get("MHA_TRACE", "0")))
    res = run_bass_kernel_spmd(nc, in_maps, core_ids=list(range(8)), trace=trace)
    last_results = res
    last_exec_time_ns = res.exec_time_ns

    out = np.zeros((B, L, DIM), np.float32)
    for core in range(8):
        b = core // 4
        out[b] += res.results[core]["out"]
    return out
